# revision 45
# baseline (speedup 1.0000x reference)
"""Relative-position attention (Shaw-style) on 8 TRN2 NeuronCores.

Sharding: interleaved sequence-parallel over query positions. Core i handles
global rows t = 8r + i (r in [0,128)) for all batches. Causality makes row r
need only keys k <= t, i.e. the first nc_r = r//16 + 1 chunks of 128 keys --
identical on every core, so one SPMD graph is load-balanced AND skips ~44% of
the E_Q/E_S table traffic and compute.

Layouts are chosen so scores live TRANSPOSED (k on partitions, query-row on
the free dim): the softmax exp then emits p^T directly and no PE transposes
or SBUF scatter DMAs are needed anywhere.

dtypes: E_Q/kT/kwT in fp8e4 (their score contributions tolerate it), v/E_S
and the probability path in bf16, accumulation f32.
"""

import numpy as np
import ml_dtypes

import concourse.bass as bass
import concourse.tile as tile
import concourse.mybir as mybir
from concourse.bass_utils import run_bass_kernel_spmd

BF16 = ml_dtypes.bfloat16
FP8 = ml_dtypes.float8_e4m3fn

B, T, D, H = 16, 1024, 256, 64
NCORES = 8
TL = T // NCORES      # 128 query rows per core
KC = T // 128         # 8 key chunks
NP = 16               # gpairs: 8 consecutive local rows each
NCH = [p // 2 + 1 for p in range(NP)]       # valid key chunks for gpair p
EQOFF = np.cumsum([0] + [8 * n * 128 for n in NCH]).tolist()
ESOFF = np.cumsum([0] + [8 * n * 64 for n in NCH]).tolist()
EQCOLS = EQOFF[-1]    # 73728
ESCOLS = ESOFF[-1]    # 36864

TRACE = False
last_bench = None
_graph_cache = None


def _build_graph(split_waits=True):
    nc = bass.Bass()
    bf = mybir.dt.bfloat16
    f32 = mybir.dt.float32
    f8 = mybir.dt.float8e4

    qT = nc.dram_tensor("qT", [B, D, TL], bf, kind="ExternalInput")
    kT8 = nc.dram_tensor("kT8", [B, D, T], f8, kind="ExternalInput")
    vT = nc.dram_tensor("vT", [B, D, T], bf, kind="ExternalInput")
    wq = nc.dram_tensor("wq", [D, H], bf, kind="ExternalInput")
    wk = nc.dram_tensor("wk", [D, H], bf, kind="ExternalInput")
    wv = nc.dram_tensor("wv", [D, H], bf, kind="ExternalInput")
    eq8 = nc.dram_tensor("eq8", [H, EQCOLS], f8, kind="ExternalInput")
    es16 = nc.dram_tensor("es16", [128, ESCOLS], bf, kind="ExternalInput")
    maskT = nc.dram_tensor("maskT", [128, T], bf, kind="ExternalInput")
    out = nc.dram_tensor("out", [B, TL, H], f32, kind="ExternalOutput")

    with tile.TileContext(nc) as tc:
        with tc.tile_pool(name="persist", bufs=1) as persist:
            # relT: transposed scores bias; col = kc*2048 + b*128 + r.
            # Pre-filled with the causal mask so phase-B in-place adds leave
            # masked/never-computed entries at -1e9.
            relT = persist.tile([128, KC * B * TL], bf, tag="relT")
            # pT: p^T per batch; col = b*1024 + kc*128 + r
            pT = persist.tile([128, B * KC * TL], bf, tag="pT")
            qw = persist.tile([H, B * TL], bf, tag="qw")        # col = b*128+r
            kwT8 = persist.tile([H, B * T], f8, tag="kwT8")     # col = b*1024+k
            # vw with a ones column appended per chunk: col = (b*KC+kc)*65 + h
            vw = persist.tile([128, B * KC * (H + 1)], bf, tag="vw")
            hacc = persist.tile([TL, B * H], f32, tag="hacc")   # col = b*64+h
            relh_alt = persist.tile([B, TL * H], bf, tag="relh_alt")  # [b,(r,h)]
            rinv = persist.tile([TL, B], f32, tag="rinv")
            msk = persist.tile([128, T], bf, tag="msk")
            osb = persist.tile([TL, B * H], f32, tag="osb")
            wq_s = persist.tile([128, 2 * H], bf, tag="wq_s")
            wk_s = persist.tile([128, 2 * H], bf, tag="wk_s")
            wv_s = persist.tile([128, 2 * H], bf, tag="wv_s")

            nc.sync.dma_start(msk[:], maskT[:, :])
            for dm in range(2):
                nc.sync.dma_start(wq_s[:, dm * H:(dm + 1) * H], wq[dm * 128:(dm + 1) * 128, :])
                nc.sync.dma_start(wk_s[:, dm * H:(dm + 1) * H], wk[dm * 128:(dm + 1) * 128, :])
                nc.sync.dma_start(wv_s[:, dm * H:(dm + 1) * H], wv[dm * 128:(dm + 1) * 128, :])

            # ones column of vw_aug (col 64 of each 65-chunk)
            vw_v = vw[:].rearrange("p (x c) -> p x c", c=H + 1)
            nc.vector.memset(vw_v[:, :, H:H + 1], 1.0)

            def ecopy(eng, dst, src):
                if eng is nc.scalar:
                    eng.copy(dst, src)
                else:
                    eng.tensor_copy(dst, src)

            def eadd(eng, dst, in0, in1):
                if eng is nc.scalar:
                    eng.add(dst, in0, in1)
                else:
                    eng.tensor_add(dst, in0, in1)

            # init relT := mask, broadcast over b (DVE/Act; Pool is 6x slower)
            relT_v = relT[:].rearrange("p (kc b r) -> p b kc r", kc=KC, b=B)
            msk_v = msk[:].rearrange("p (kc r) -> p kc r", kc=KC)
            for b in range(B):
                ecopy((nc.vector, nc.scalar)[b % 2], relT_v[:, b], msk_v)

            qw_v = qw[:].rearrange("d (b r) -> d r b", r=TL)    # [64, TL, B]
            pT_v = pT[:].rearrange("p (b x) -> p x b", b=B)     # [128, 1024, B]

            # ---------- A0: q projection ----------
            with tc.tile_pool(name="phA0", bufs=2) as phA0, \
                 tc.tile_pool(name="psA0", bufs=2, space="PSUM") as psA0:
                for bp in range(B // 4):
                    qt = phA0.tile([128, 4 * 2 * TL], bf, tag="qt")
                    nc.gpsimd.dma_start(
                        qt[:].rearrange("p (x r) -> p x r", x=8),
                        qT[4 * bp:4 * bp + 4, :, :].rearrange(
                            "g (dm p) r -> p (g dm) r", p=128))
                    for g in range(4):
                        b = 4 * bp + g
                        psq = psA0.tile([H, TL], f32, tag="psq")
                        for dm in range(2):
                            nc.tensor.matmul(
                                psq[:],
                                lhsT=wq_s[:, dm * H:(dm + 1) * H],
                                rhs=qt[:, (2 * g + dm) * TL:(2 * g + dm + 1) * TL],
                                start=(dm == 0), stop=(dm == 1))
                        nc.vector.tensor_copy(qw[:, b * TL:(b + 1) * TL], psq[:])

            # ---------- B: rel_q bmm (eq stationary), k/v projection mixed in ----------
            with tc.tile_pool(name="phB", bufs=3) as phB, \
                 tc.tile_pool(name="phA1", bufs=3) as phA1, \
                 tc.tile_pool(name="psB", bufs=2, space="PSUM") as psB, \
                 tc.tile_pool(name="psK", bufs=1, space="PSUM") as psK, \
                 tc.tile_pool(name="psV", bufs=2, space="PSUM") as psV:

                def a1_body(b):
                    kt = phA1.tile([128, 2 * T], f8, tag="kt", name="kt")
                    nc.scalar.dma_start(
                        kt[:].rearrange("p (dm k) -> p dm k", dm=2),
                        kT8[b, :, :].rearrange("(dm p) k -> p dm k", p=128))
                    psk = psK.tile([H, T], f32, tag="psk")
                    for h2 in range(2):
                        for dm in range(2):
                            nc.tensor.matmul(
                                psk[:, h2 * 512:(h2 + 1) * 512],
                                lhsT=wk_s[:, dm * H:(dm + 1) * H],
                                rhs=kt[:, dm * T + h2 * 512:dm * T + (h2 + 1) * 512],
                                start=(dm == 0), stop=(dm == 1))
                    nc.vector.tensor_copy(kwT8[:, b * T:b * T + 512], psk[:, 0:512])
                    nc.scalar.copy(kwT8[:, b * T + 512:(b + 1) * T], psk[:, 512:1024])

                    vt = phA1.tile([128, 2 * T], bf, tag="vt", name="vt")
                    nc.scalar.dma_start(
                        vt[:].rearrange("p (dm k) -> p dm k", dm=2),
                        vT[b, :, :].rearrange("(dm p) k -> p dm k", p=128))
                    psv = psV.tile([128, KC * H], f32, tag="psv")
                    for kc in range(KC):
                        for dm in range(2):
                            nc.tensor.matmul(
                                psv[:, kc * H:(kc + 1) * H],
                                lhsT=vt[:, dm * T + kc * 128:dm * T + (kc + 1) * 128],
                                rhs=wv_s[:, dm * H:(dm + 1) * H],
                                start=(dm == 0), stop=(dm == 1))
                    # one evac per b: dest strided over the 65-wide chunks
                    nc.scalar.copy(
                        vw_v[:, b * KC:(b + 1) * KC, 0:H],
                        psv[:].rearrange("p (kc h) -> p kc h", kc=KC))

                relT_e = relT[:].rearrange(
                    "p (kc b r) -> p kc b r", kc=KC, b=B)
                for p in range(NP):
                    n = NCH[p]
                    eqt = phB.tile([H, 8 * n * 128], f8, tag="eqt")
                    nc.sync.dma_start(eqt[:], eq8[:, EQOFF[p]:EQOFF[p + 1]])
                    for q in range(2):       # 4-row batches
                        prT = psB.tile([128, 4 * n * B], f32, tag="prT")
                        for j4 in range(4):
                            r = 8 * p + 4 * q + j4
                            for kc in range(n):
                                nc.tensor.matmul(
                                    prT[:, (j4 * n + kc) * B:(j4 * n + kc + 1) * B],
                                    lhsT=eqt[:, ((4 * q + j4) * n + kc) * 128:
                                             ((4 * q + j4) * n + kc + 1) * 128],
                                    rhs=qw_v[:, r, :],
                                    start=True, stop=True)
                        # in-place add onto mask-initialized relT (4 rows at once)
                        r0 = 8 * p + 4 * q
                        dst = relT_e[:, 0:n, :, r0:r0 + 4]
                        src = prT[:].rearrange(
                            "p (j kc b) -> p kc b j", j=4, kc=n)
                        nc.vector.tensor_add(dst, src, dst)
                    a1_body(p)

            # ---------- C: transposed scores + softmax + content heads ----------
            with tc.tile_pool(name="phC", bufs=4) as phC, \
                 tc.tile_pool(name="psC", bufs=2, space="PSUM") as psC, \
                 tc.tile_pool(name="psH", bufs=2, space="PSUM") as psH:
                for b in range(B):
                    sT = psC.tile([128, T], f32, tag="sT")
                    for kc in range(KC):
                        nc.tensor.matmul(
                            sT[:, kc * 128:(kc + 1) * 128],
                            lhsT=kwT8[:, b * T + kc * 128:b * T + (kc + 1) * 128],
                            rhs=qw[:, b * TL:(b + 1) * TL],
                            start=True, stop=True)
                    ssbT = phC.tile([128, T], bf, tag="ssbT")
                    nc.vector.tensor_add(
                        ssbT[:].rearrange("p (kc r) -> p kc r", kc=KC),
                        sT[:].rearrange("p (kc r) -> p kc r", kc=KC),
                        relT_v[:, b])
                    nc.scalar.activation(pT[:, b * T:(b + 1) * T], ssbT[:],
                                         mybir.ActivationFunctionType.Exp)
                    psh = psH.tile([TL, H + 1], f32, tag="psh")
                    for kc in range(KC):
                        nc.tensor.matmul(
                            psh[:],
                            lhsT=pT[:, b * T + kc * 128:b * T + (kc + 1) * 128],
                            rhs=vw[:, (b * KC + kc) * (H + 1):(b * KC + kc + 1) * (H + 1)],
                            start=(kc == 0), stop=(kc == KC - 1))
                    nc.scalar.copy(hacc[:, b * H:(b + 1) * H], psh[:, 0:H])
                    nc.vector.reciprocal(rinv[:, b:b + 1], psh[:, H:H + 1])

            # ---------- D: rel heads = p . E_S ----------
            with tc.tile_pool(name="phD", bufs=6) as phD, \
                 tc.tile_pool(name="psD", bufs=2, space="PSUM") as psD:
                for p in range(NP):
                    n = NCH[p]
                    est = phD.tile([128, 8 * n * H], bf, tag="est")
                    nc.sync.dma_start(est[:], es16[:, ESOFF[p]:ESOFF[p + 1]])
                    prh = psD.tile([B, 8 * H], f32, tag="prh")
                    for j in range(8):
                        r = 8 * p + j
                        for kc in range(n):
                            nc.tensor.matmul(
                                prh[:, j * H:(j + 1) * H],
                                lhsT=pT_v[:, kc * 128 + r, :],
                                rhs=est[:, (j * n + kc) * H:(j * n + kc + 1) * H],
                                start=(kc == 0), stop=(kc == n - 1))
                    eng = (nc.vector, nc.scalar)[p % 2]
                    ecopy(eng, relh_alt[:, p * 8 * H:(p + 1) * 8 * H], prh[:])

            # ---------- E: combine + normalize + store ----------
            with tc.tile_pool(name="phE", bufs=4) as phE:
                for b in range(B):
                    rstage = phE.tile([TL, H], bf, tag="rstage")
                    (nc.sync if b % 2 == 0 else nc.scalar).dma_start(
                        rstage[:], relh_alt[b:b + 1, :])
                    nc.vector.tensor_add(osb[:, b * H:(b + 1) * H],
                                         hacc[:, b * H:(b + 1) * H],
                                         rstage[:])
                    nc.vector.tensor_scalar_mul(osb[:, b * H:(b + 1) * H],
                                                osb[:, b * H:(b + 1) * H],
                                                rinv[:, b:b + 1])
                nc.sync.dma_start(
                    out[:, :, :].rearrange("b r h -> r b h"),
                    osb[:].rearrange("r (b h) -> r b h", b=B))

    if split_waits:
        _split_dma_waits(nc)
    return nc


def _split_dma_waits(nc):
    """walrus's instruction encodings carry at most ONE sem wait; Tile can
    emit several. Hoist extra waits onto standalone EventSemaphore ops."""
    wid = [0]
    for f in nc.m.functions:
        for blk in f.blocks:
            il = blk.instructions
            i = 0
            while i < len(il):
                inst = il[i]
                si = getattr(inst, "sync_info", None)
                if (si is not None and len(si.on_wait) > 1
                        and inst.opcode != "EventSemaphore"):
                    for w in si.on_wait:
                        ev = mybir.InstEventSemaphore(
                            name=f"WSPLIT-{wid[0]}", ins=[], outs=[])
                        wid[0] += 1
                        ev.engine = inst.engine
                        ev.sync_info = mybir.SyncInfo(on_wait=[w], on_update=[])
                        il.insert(i, ev)
                        i += 1
                    inst.sync_info = mybir.SyncInfo(
                        on_wait=[], on_update=list(si.on_update))
                i += 1


def kernel(query, value, key, W_Q, W_V, W_K, alpha, E_Q, E_S):
    global _graph_cache, last_bench
    query = np.asarray(query, np.float32)
    value = np.asarray(value, np.float32)
    key = np.asarray(key, np.float32)
    W_Q = np.asarray(W_Q, np.float32)
    W_V = np.asarray(W_V, np.float32)
    W_K = np.asarray(W_K, np.float32)
    alpha = np.asarray(alpha, np.float32)
    E_Q = np.asarray(E_Q, np.float32)
    E_S = np.asarray(E_S, np.float32)

    # fold alpha / sqrt(D) into query
    q_scaled = query * (alpha / 8.0)[None, :, :]                    # [B,T,D]
    kT8_full = np.ascontiguousarray(key.transpose(0, 2, 1)).astype(FP8)
    vT_full = np.ascontiguousarray(value.transpose(0, 2, 1)).astype(BF16)
    wq_b = W_Q.astype(BF16)
    wk_b = W_K.astype(BF16)
    wv_b = W_V.astype(BF16)
    karange = np.arange(T)

    in_maps = []
    for i in range(NCORES):
        trange = np.arange(i, T, NCORES)                            # t = 8r+i
        qT_i = np.ascontiguousarray(
            q_scaled[:, i::NCORES, :].transpose(0, 2, 1)).astype(BF16)
        # eq8: per row r keep valid chunks, transposed to [d, k]
        eqc = np.ascontiguousarray(
            E_Q[i::NCORES].transpose(0, 2, 1)).astype(FP8)          # [TL,64,T]
        eq_flat = np.concatenate(
            [eqc[r][:, :(r // 16 + 1) * 128] for r in range(TL)],
            axis=1)                                                 # [64,EQCOLS]
        # es16: per row r keep valid chunks as [128 kp, n*64]
        esc = E_S[i::NCORES]                                        # [TL,T,64]
        es_flat = np.concatenate(
            [esc[r, :(r // 16 + 1) * 128, :].reshape(r // 16 + 1, 128, 64)
             .transpose(1, 0, 2).reshape(128, -1) for r in range(TL)],
            axis=1).astype(BF16)                                    # [128,ESCOLS]
        # maskT[kp, kc*128+r] = -1e9 where k=kc*128+kp > t=8r+i
        kgrid = karange.reshape(KC, 128).T                          # [128 kp, kc]
        mT = np.where(kgrid[:, :, None] > trange[None, None, :],
                      -1e9, 0.0).reshape(128, KC * TL).astype(BF16)
        in_maps.append({
            "qT": qT_i,
            "kT8": kT8_full,
            "vT": vT_full,
            "wq": wq_b, "wk": wk_b, "wv": wv_b,
            "eq8": eq_flat,
            "es16": es_flat,
            "maskT": mT,
        })

    if _graph_cache is None:
        _graph_cache = _build_graph()

    res = run_bass_kernel_spmd(_graph_cache, in_maps,
                               core_ids=list(range(NCORES)), trace=TRACE)
    last_bench = res
    full = np.empty((B, T, H), np.float32)
    for i in range(NCORES):
        full[:, i::NCORES, :] = res.results[i]["out"]
    return full
